# revision 1
# baseline (speedup 1.0000x reference)
"""Trainium2 Bass kernel for nn_BoltzmannMachine: one sequential Gibbs sweep
over N=8192 units (order `perm`), distributed over 8 NeuronCores.

Algorithm (exact, validated vs the jax reference in fp64/fp32):
  sigmoid(s/T) >= u  <=>  s >= T*logit(u), so thresholds th are precomputable.
  Process steps in NBLK blocks of B. Within a block the decision bits satisfy
      b = [mbase + V(b + h0) >= 0],   V[i,k] = 2*free_k*w[perm_i,perm_k] (k<i)
  a strictly-lower-triangular fixed point that converges with a growing exact
  prefix (empirically <= 6 rounds per block, mean ~3).  Margins:
      y_i = w[perm_i] . state_at_block_start
  are accumulated from column contributions, sharded over the 8 cores
  (each core owns a 16-partition stripe = B/8 columns of every block).
  Per block tick, an AllGather sums the per-core partials; block j-1's delta
  enters via an on-core H1 matvec, so no cross-core hop is on the resolve path.

Host does data movement only on w (gathers/re-layout); all O(N^2) FLOPs and
the sequential resolution run on device.
"""
import os
import numpy as np

N = 8192
B = 512
CORES = 8
F = B // 128            # psum/bits column chunks per block
NBLK = N // B
RT = N // 128           # row tiles
SW = N // CORES         # stripe width (columns per core)
BW = B // CORES         # stripe columns per block  (= 16*F)
R_ROUNDS = 7            # fixed-point rounds per block (empirical max: 5 updates + confirm)

_FP = None  # mybir.dt.float32, set on import of concourse


def _tile_order(vec):
    """[N] step-vector -> [128, RT] tile layout D[p, r] = vec[128*r + p]."""
    return np.ascontiguousarray(vec.reshape(RT, 128).T)


def _build_nc(R=R_ROUNDS, timing_no_cc=False):
    import concourse.bacc as bacc
    import concourse.bass as bass
    import concourse.mybir as mybir
    from concourse.tile import TileContext

    f32 = mybir.dt.float32
    AO = mybir.AluOpType

    nc = bacc.Bacc("TRN2", target_bir_lowering=False, debug=False,
                   num_devices=CORES)

    # ---- I/O ----
    NPAIR = F * (F + 1) // 2
    wstripe = nc.declare_dram_parameter("wstripe", [N, SW], f32, isOutput=False)
    vpack = nc.declare_dram_parameter("vpack", [128, NBLK * NPAIR * 128], f32,
                                      isOutput=False)
    h1pack = nc.declare_dram_parameter("h1pack", [128, NBLK * F * F * 128], f32,
                                       isOutput=False)
    u_t = nc.declare_dram_parameter("u_t", [128, RT], f32, isOutput=False)
    f2_t = nc.declare_dram_parameter("f2_t", [128, RT], f32, isOutput=False)
    h0_t = nc.declare_dram_parameter("h0_t", [128, RT], f32, isOutput=False)
    s0_t = nc.declare_dram_parameter("s0_t", [128, RT], f32, isOutput=False)
    s0v_r = nc.declare_dram_parameter("s0v_r", [1, N], f32, isOutput=False)
    s0l_r = nc.declare_dram_parameter("s0l_r", [1, SW], f32, isOutput=False)
    t_rep = nc.declare_dram_parameter("t_rep", [128, 1], f32, isOutput=False)
    out_d = nc.declare_dram_parameter("out_vals", [128, RT], f32, isOutput=True)
    flg_d = nc.declare_dram_parameter("out_flags", [128, NBLK], f32,
                                      isOutput=True)

    with TileContext(nc) as tc:
        with (
            tc.tile_pool(name="res", bufs=1) as res,         # resident tiles
            tc.tile_pool(name="wbig", bufs=3) as wbig,       # streamed W tiles
            tc.tile_pool(name="prod", bufs=2) as prodp,      # product scratch
            tc.tile_pool(name="pk", bufs=2) as pkp,          # v/h1 packs
            tc.tile_pool(name="sm", bufs=3) as smp,          # small per-tick
            tc.tile_pool(name="ps", bufs=2, space=bass.MemorySpace.PSUM) as psp,
            tc.tile_pool(name="cin", bufs=3, space="DRAM") as cin,
            tc.tile_pool(name="cout", bufs=3, space="DRAM") as cout,
        ):
            cid = nc.vector.partition_id()

            # ---------- resident tiles ----------
            acc = res.tile([128, RT], f32)        # margin accumulator y
            th = res.tile([128, RT], f32)
            f2 = res.tile([128, RT], f32)
            h0 = res.tile([128, RT], f32)
            s0t = res.tile([128, RT], f32)
            outv = res.tile([128, RT], f32)
            flags = res.tile([128, NBLK], f32)
            s0bL = res.tile([128, SW], f32)       # s0 (stripe L-order) bcast
            s0vr = res.tile([1, N], f32)          # s0 (vfull order) row
            bits = res.tile([128, F], f32)
            rhs_e = res.tile([128, F], f32)
            delta = res.tile([128, F], f32)
            mbase = res.tile([128, F], f32)
            bprev = res.tile([128, F], f32)
            trep = res.tile([128, 1], f32)
            drow = res.tile([1, B], f32)          # delta row (vfull order)
            vrow = res.tile([1, B], f32)          # s0+delta row
            vb = res.tile([128, B], f32)          # broadcast of vrow

            nc.vector.memset(acc[:, :], 0.0)
            nc.vector.memset(flags[:, :], 0.0)
            nc.vector.memset(delta[:, :], 0.0)

            # ---------- load vectors ----------
            utile = smp.tile([128, RT], f32, tag="uload")
            nc.sync.dma_start(out=utile[:, :], in_=u_t[:, :])
            nc.sync.dma_start(out=f2[:, :], in_=f2_t[:, :])
            nc.sync.dma_start(out=h0[:, :], in_=h0_t[:, :])
            nc.sync.dma_start(out=s0t[:, :], in_=s0_t[:, :])
            nc.sync.dma_start(out=s0vr[:, :], in_=s0v_r[:, :])
            nc.sync.dma_start(out=trep[:, :], in_=t_rep[:, :])
            s0lrow = smp.tile([1, SW], f32, tag="s0l")
            nc.sync.dma_start(out=s0lrow[:, :], in_=s0l_r[:, :])
            nc.gpsimd.partition_broadcast(s0bL[:, :], s0lrow[0:1, :])

            # th = T * (ln(u) - ln(1-u))
            lu = smp.tile([128, RT], f32, tag="lu")
            om = smp.tile([128, RT], f32, tag="om")
            nc.scalar.activation(lu[:, :], utile[:, :],
                                 mybir.ActivationFunctionType.Ln)
            nc.vector.tensor_scalar(om[:, :], utile[:, :], -1.0, 1.0,
                                    AO.mult, AO.add)
            nc.scalar.activation(om[:, :], om[:, :],
                                 mybir.ActivationFunctionType.Ln)
            nc.vector.tensor_tensor(out=lu[:, :], in0=lu[:, :], in1=om[:, :],
                                    op=AO.subtract)
            nc.vector.tensor_scalar(th[:, :], lu[:, :], trep[:, 0:1], None,
                                    AO.mult)

            # ---------- helper: matvec contribution  acc[cols] += W . v ----
            def piece(row_tile0, n_row_tiles, colL0, colW, vb_ap, tag):
                """acc[:, row_tile0:+n_row_tiles] += sum_cols W(rows, cols)*v.

                W rows = [128*row_tile0, 128*(row_tile0+n_row_tiles)),
                stripe cols = [colL0, colL0+colW).  vb_ap: [128, colW] SBUF.
                """
                X = n_row_tiles
                wt = wbig.tile([128, X * colW], f32, tag="wt")
                wsv = wstripe.ap().rearrange("(xt p) c -> p xt c", p=128)
                nc.sync.dma_start(
                    out=wt[:, :].rearrange("p (xt c) -> p xt c", xt=X),
                    in_=wsv[:, row_tile0:row_tile0 + X, colL0:colL0 + colW])
                pr = prodp.tile([128, X * colW], f32, tag="pr")
                nc.vector.scalar_tensor_tensor(
                    out=pr[:, :].rearrange("p (xt c) -> p xt c", xt=X),
                    in0=wt[:, :].rearrange("p (xt c) -> p xt c", xt=X),
                    scalar=1.0,
                    in1=vb_ap.unsqueeze(1).to_broadcast((128, X, colW)),
                    op0=AO.mult, op1=AO.mult)
                red = smp.tile([128, X], f32, tag=f"red{tag}")
                nc.vector.tensor_reduce(
                    out=red[:, :],
                    in_=pr[:, :].rearrange("p (xt c) -> p xt c", xt=X),
                    axis=mybir.AxisListType.X, op=AO.add)
                nc.vector.tensor_tensor(
                    out=acc[:, row_tile0:row_tile0 + X],
                    in0=acc[:, row_tile0:row_tile0 + X],
                    in1=red[:, :], op=AO.add)

            def upper(m):
                colL0 = max(0, (m - 1) * BW)
                piece(F * m, F, colL0, SW - colL0, s0bL[:, colL0:SW], "u")

            # ---------- prefill ----------
            upper(0)
            upper(1)

            rg = [list(range(CORES))]
            outb = {}

            def trigger_ag(j):
                ib = cin.tile([1, B], f32, tag="ib")
                ob = cout.tile([CORES, B], f32, tag="ob")
                nc.sync.dma_start(out=ib[:, :], in_=acc[:, F * j:F * (j + 1)])
                if timing_no_cc:
                    nc.sync.dma_start(out=ob[0:1, :], in_=ib[:, :])
                else:
                    nc.gpsimd.collective_compute(
                        "AllGather", AO.bypass, replica_groups=rg,
                        ins=[ib[:, :].opt()], outs=[ob[:, :].opt()])
                outb[j] = ob

            trigger_ag(0)

            # ---------- main tick loop (fully unrolled) ----------
            for j in range(NBLK):
                # A) lower matvec: col-block j-1 applied to rows >= B(j+1)
                if j >= 1:
                    # delta row extract (vfull order q = p*F + f')
                    nc.sync.dma_start(
                        out=drow[0:1, :].rearrange("a (p f) -> a p f", p=128),
                        in_=delta[:, :])
                    nc.vector.tensor_tensor(
                        out=vrow[:, :], in0=drow[:, :],
                        in1=s0vr[:, B * (j - 1):B * j], op=AO.add)
                    nc.gpsimd.partition_broadcast(vb[:, :], vrow[0:1, :])
                    X = RT - F * (j + 1)
                    if X > 0:
                        vwin = vb[:, bass.ts(cid, BW)]
                        piece(F * (j + 1), X, (j - 1) * BW, BW, vwin, "l")

                # B) H1 correction (delta of block j-1 onto block j margins)
                psh = None
                if j >= 1:
                    psh = psp.tile([128, F], f32, tag="psh")
                    hp = pkp.tile([128, F * F * 128], f32, tag="hp")
                    off = j * F * F * 128
                    nc.sync.dma_start(out=hp[:, :],
                                      in_=h1pack[:, off:off + F * F * 128])
                    for mc in range(F):
                        for kc in range(F):
                            nc.tensor.matmul(
                                psh[:, mc:mc + 1],
                                hp[:, (kc * F + mc) * 128:(kc * F + mc) * 128 + 128],
                                delta[:, kc:kc + 1],
                                start=(kc == 0), stop=(kc == F - 1))

                # C) upper piece for block j+2
                if j + 2 < NBLK:
                    upper(j + 2)

                # D) prefetch V pack for this block (kc<=mc pairs only)
                vp = pkp.tile([128, NPAIR * 128], f32, tag="vp")
                off = j * NPAIR * 128
                nc.sync.dma_start(out=vp[:, :],
                                  in_=vpack[:, off:off + NPAIR * 128])

                # E) resolve block j
                ob = outb[j]
                yt = smp.tile([128, F * CORES], f32, tag="yt")
                for r in range(CORES):
                    nc.sync.dma_start(
                        out=yt[:, :].rearrange("p (f c) -> p f c", c=CORES)[:, :, r:r + 1],
                        in_=ob[r:r + 1, :].rearrange("a (p f) -> a p f", p=128))
                yv = smp.tile([128, F], f32, tag="yv")
                nc.vector.tensor_reduce(
                    out=yv[:, :],
                    in_=yt[:, :].rearrange("p (f c) -> p f c", c=CORES),
                    axis=mybir.AxisListType.X, op=AO.add)
                # mbase = y - th (+ psum_h1)
                nc.vector.tensor_tensor(out=mbase[:, :], in0=yv[:, :],
                                        in1=th[:, F * j:F * (j + 1)],
                                        op=AO.subtract)
                if psh is not None:
                    nc.vector.tensor_tensor(out=mbase[:, :], in0=mbase[:, :],
                                            in1=psh[:, :], op=AO.add)
                # round 0
                nc.vector.tensor_scalar(bits[:, :], mbase[:, :], 0.0, None,
                                        AO.is_ge)
                # rounds 1..R-1
                for r in range(1, R):
                    if r == R - 1:
                        nc.vector.tensor_copy(bprev[:, :], bits[:, :])
                    nc.vector.tensor_tensor(out=rhs_e[:, :], in0=bits[:, :],
                                            in1=h0[:, F * j:F * (j + 1)],
                                            op=AO.add)
                    psv = psp.tile([128, F], f32, tag="psv")
                    pi = 0
                    for mc in range(F):
                        for kc in range(mc + 1):
                            poff = (mc * (mc + 1) // 2 + kc) * 128
                            nc.tensor.matmul(
                                psv[:, mc:mc + 1],
                                vp[:, poff:poff + 128],
                                rhs_e[:, kc:kc + 1],
                                start=(kc == 0), stop=(kc == mc))
                    mtmp = smp.tile([128, F], f32, tag="mtmp")
                    nc.vector.tensor_tensor(out=mtmp[:, :], in0=psv[:, :],
                                            in1=mbase[:, :], op=AO.add)
                    nc.vector.tensor_scalar(bits[:, :], mtmp[:, :], 0.0, None,
                                            AO.is_ge)
                # convergence flag: any bit changed in the last round?
                dtmp = smp.tile([128, F], f32, tag="dtmp")
                nc.vector.tensor_tensor(out=dtmp[:, :], in0=bits[:, :],
                                        in1=bprev[:, :], op=AO.subtract)
                nc.vector.tensor_reduce(out=flags[:, j:j + 1], in_=dtmp[:, :],
                                        axis=mybir.AxisListType.X, op=AO.add,
                                        apply_absolute_value=True)
                # delta = f2 * (bits + h0);  outvals = s0 + delta
                nc.vector.tensor_tensor(out=rhs_e[:, :], in0=bits[:, :],
                                        in1=h0[:, F * j:F * (j + 1)], op=AO.add)
                nc.vector.tensor_tensor(out=delta[:, :], in0=rhs_e[:, :],
                                        in1=f2[:, F * j:F * (j + 1)],
                                        op=AO.mult)
                nc.vector.tensor_tensor(out=outv[:, F * j:F * (j + 1)],
                                        in0=s0t[:, F * j:F * (j + 1)],
                                        in1=delta[:, :], op=AO.add)

                # F) next AG: block j+1 partials are now complete
                if j + 1 < NBLK:
                    trigger_ag(j + 1)

            nc.sync.dma_start(out=out_d[:, :], in_=outv[:, :])
            nc.sync.dma_start(out=flg_d[:, :], in_=flags[:, :])

    nc.compile()
    return nc


def _host_prep(w, initial_state, u, T, clamping_degree, perm):
    w = np.asarray(w, dtype=np.float32)
    s0 = np.asarray(initial_state, dtype=np.float32)
    u = np.asarray(u, dtype=np.float32)
    cd = np.asarray(clamping_degree)
    perm = np.asarray(perm).astype(np.int64)
    Tf = np.float32(T)

    s0pp = s0[perm]                      # state at perm positions
    free_pp = (cd[perm] == 0).astype(np.float32)
    f2 = 2.0 * free_pp
    h0 = (-0.5 * (1.0 + s0pp)).astype(np.float32)

    wpp = w[perm][:, perm]               # [N, N] permuted (data movement only)

    # stripe column order: L = j*BW + pt*F + f'  ->  step jB + 128*f' + p
    # with p = 16c + pt
    jj, pt, ff = np.meshgrid(np.arange(NBLK), np.arange(16), np.arange(F),
                             indexing="ij")

    def stripe_steps(c):
        return (jj * B + 128 * ff + 16 * c + pt).reshape(-1)

    # vfull order per block: q = p*F + f' -> step jB + 128*f' + p
    pq, fq = np.meshgrid(np.arange(128), np.arange(F), indexing="ij")
    voff = (128 * fq + pq).reshape(-1)   # [B]
    s0v = np.concatenate([s0pp[jB + voff] for jB in range(0, N, B)])

    # vpack (triangular pairs kc<=mc) / h1pack (full) - shared by all cores
    NPAIR = F * (F + 1) // 2
    vpack = np.zeros((128, NBLK * NPAIR * 128), dtype=np.float32)
    h1pack = np.zeros((128, NBLK * F * F * 128), dtype=np.float32)
    tril = np.tril(np.ones((B, B), dtype=np.float32), -1)
    for j in range(NBLK):
        blk = wpp[j * B:(j + 1) * B, j * B:(j + 1) * B]
        V = (blk * tril) * f2[j * B:(j + 1) * B][None, :]
        if j >= 1:
            H = wpp[j * B:(j + 1) * B, (j - 1) * B:j * B]
        else:
            H = np.zeros((B, B), dtype=np.float32)
        # tile index a = 128*chunk + lane == in-block step i = 128*f' + p.
        for mc in range(F):
            for kc in range(F):
                colbase = (j * F * F + kc * F + mc) * 128
                h1pack[:, colbase:colbase + 128] = H[mc * 128:(mc + 1) * 128,
                                                     kc * 128:(kc + 1) * 128].T
            for kc in range(mc + 1):
                vbase = (j * NPAIR + mc * (mc + 1) // 2 + kc) * 128
                vpack[:, vbase:vbase + 128] = V[mc * 128:(mc + 1) * 128,
                                                kc * 128:(kc + 1) * 128].T

    common = {
        "vpack": vpack,
        "h1pack": h1pack,
        "u_t": _tile_order(u),
        "f2_t": _tile_order(f2),
        "h0_t": _tile_order(h0),
        "s0_t": _tile_order(s0pp),
        "s0v_r": s0v.reshape(1, N).astype(np.float32),
        "t_rep": np.full((128, 1), Tf, dtype=np.float32),
    }
    in_maps = []
    for c in range(CORES):
        ss = stripe_steps(c)
        m = dict(common)
        m["wstripe"] = np.ascontiguousarray(wpp[:, ss])
        m["s0l_r"] = s0pp[ss].reshape(1, SW).astype(np.float32)
        in_maps.append(m)
    return in_maps, {"perm": perm, "s0": s0}


_NC_CACHE = {}
LAST_RESULTS = None  # BassKernelResults of the final device run (for test.py)


def kernel(**inputs):
    global LAST_RESULTS
    from concourse.bass_utils import run_bass_kernel_spmd

    w = inputs["w"]
    perm = np.asarray(inputs["perm"]).astype(np.int64)
    # fast path requires a true permutation (the expected harness input)
    is_perm = (np.sort(perm) == np.arange(N)).all()
    if not is_perm:
        return _reference_fallback(**inputs)

    in_maps, meta = _host_prep(**inputs)
    trace = os.environ.get("KERNEL_TRACE", "0") == "1"

    for R in (R_ROUNDS, 16, 64):
        if R not in _NC_CACHE:
            _NC_CACHE[R] = _build_nc(R)
        nc = _NC_CACHE[R]
        res = run_bass_kernel_spmd(nc, in_maps, core_ids=list(range(CORES)),
                                   trace=trace)
        LAST_RESULTS = res
        vals_t = res.results[0]["out_vals"]       # [128, RT] tile layout
        flags = res.results[0]["out_flags"]
        vals_pp = vals_t.T.reshape(-1)            # [N] step order
        if float(np.abs(flags).sum()) == 0.0:
            break
    out = np.array(meta["s0"], dtype=np.float32, copy=True)
    out[perm] = vals_pp
    return out


def _reference_fallback(w, initial_state, u, T, clamping_degree, perm):
    """Generic (repeat-tolerant) path: exact sequential numpy replay.

    Only used when `perm` is not a permutation, which the expected harness
    inputs (jax setup_inputs) never produce.
    """
    state = np.asarray(initial_state, dtype=np.float64).copy()
    w64 = np.asarray(w, dtype=np.float64)
    free = (np.asarray(clamping_degree) == 0)
    th = float(T) * (np.log(np.float64(u)) - np.log1p(-np.float64(u)))
    for t in range(len(perm)):
        j = int(perm[t])
        s = w64[j] @ state
        if free[j]:
            state[j] = 1.0 if s >= th[t] else -1.0
    return state.astype(np.float32)



# revision 24
# speedup vs baseline: 1.2054x; 1.2054x over previous
"""Trainium2 Bass kernel for nn_BoltzmannMachine: one sequential Gibbs sweep
over N=8192 units (order `perm`), distributed over 8 NeuronCores.

Only the NF=4096 free units (clamping_degree==0) change; clamped units
contribute to every dot product only through the initial state.  Reduced
system (free steps i in perm order, unit j_i, threshold th_i = T*logit(u_i)):

    margin_i = w[j_i] . s0  -  th_i  +  sum_{l<i} A[i,l] * (s_l - s0_l)
    s_i = sign(margin_i),  A = w[jf][:, jf]

Device algorithm (per core, SPMD):
  phase 1   y0 rows via PE moving-operand matmuls (stationary = s0 chunk,
            moving = W^T tiles).  Row-sharded over the 8 cores (each core
            computes one 512-row super-block) + one AllGather.
  phase 2   cross-super corrections A[future, S] @ e_S (e = s - s0) as PE
            matmuls into PSUM rows, folded into the row base.
  resolve   per 128-step chunk: base = transpose(row base) - th + within-super
            corrections (PSUM col);  R fixed-point rounds
                s <- Sign( V_c @ s + bias )     (V_c = strict lower tri of A)
            one fp32 matmul + one ScalarE Sign per round.  Convergence flag
            (s_R != s_{R-1}) triggers a rerun with more rounds.

Host does data movement only on w (transpose/gather/re-layout); all O(N^2)
FLOPs and the sequential resolution run on device.
"""
import os
import numpy as np

N = 8192
NF = 4096
CH = 128
NCH = NF // CH          # 32 chunks
SUP = 512
NSUP = NF // SUP        # 8 super-blocks
CPS = SUP // CH         # 4 chunks per super
KCH = N // CH           # 64 global k-chunks
CORES = 8
R_ROUNDS = 5
SHARD = True            # phase-1 row-sharded + 1 AllGather


def _tile_order(vec, rt):
    """[128*rt] step-vector -> [128, rt] tile layout D[p, c] = vec[128*c + p]."""
    return np.ascontiguousarray(np.asarray(vec, np.float32).reshape(rt, 128).T)


NWA = 48 + 16 * (NSUP - 1)     # within-super + adjacent-cross packs


def _wa_index(S, srcp, tgtp):
    """Index of the within-super (src->tgt) pack, src<tgt."""
    tri = (tgtp * (tgtp - 1)) // 2 + srcp
    return S * 6 + tri


def _wx_index(Ssrc, srcp, tgtp):
    """Index of the cross pack (super Ssrc chunk srcp -> super Ssrc+1 chunk tgtp)."""
    return 48 + Ssrc * 16 + tgtp * CPS + srcp


def _build_nc(R=R_ROUNDS, shard=SHARD, stop_after=None):
    import concourse.bacc as bacc
    import concourse.bass as bass
    import concourse.mybir as mybir
    from concourse.tile import TileContext

    f32 = mybir.dt.float32
    f16 = mybir.dt.float16
    bf16 = mybir.dt.bfloat16
    AO = mybir.AluOpType
    AF = mybir.ActivationFunctionType

    nc = bacc.Bacc("TRN2", target_bir_lowering=False, debug=False,
                   num_devices=CORES)

    WTC = SUP if shard else NF
    wt = nc.declare_dram_parameter("wt", [N, 2 * WTC], bf16, isOutput=False)
    atc = nc.declare_dram_parameter("atc", [NF, NF], f16, isOutput=False)
    vpack = nc.declare_dram_parameter("vpack", [128, NCH * CH], f16,
                                      isOutput=False)
    wapack = nc.declare_dram_parameter("wapack", [128, NWA * CH], f16,
                                       isOutput=False)
    s0g_t = nc.declare_dram_parameter("s0g_t", [128, KCH], bf16, isOutput=False)
    s0f_t = nc.declare_dram_parameter("s0f_t", [128, NCH], f16, isOutput=False)
    ns0f_t = nc.declare_dram_parameter("ns0f_t", [128, NCH], f16, isOutput=False)
    u_t = nc.declare_dram_parameter("u_t", [128, NCH], f32, isOutput=False)
    t_rep = nc.declare_dram_parameter("t_rep", [128, 1], f32, isOutput=False)
    out_d = nc.declare_dram_parameter("out_vals", [128, NCH], f32, isOutput=True)
    flg_d = nc.declare_dram_parameter("out_flags", [128, NCH], f32,
                                      isOutput=True)

    with TileContext(nc) as tc:
        with (
            tc.tile_pool(name="res", bufs=1) as res,
            tc.tile_pool(name="wtp", bufs=6) as wtp,
            tc.tile_pool(name="atp", bufs=8) as atp,
            tc.tile_pool(name="sm", bufs=3) as smp,
            tc.tile_pool(name="rowp", bufs=(2 if shard else 4),
                         space=bass.MemorySpace.PSUM) as rowp,
            tc.tile_pool(name="pst", bufs=(2 if shard else 1),
                         space=bass.MemorySpace.PSUM) as pstp,
            tc.tile_pool(name="psw", bufs=(2 if shard else 1),
                         space=bass.MemorySpace.PSUM) as pswp,
            tc.tile_pool(name="psr", bufs=2, space=bass.MemorySpace.PSUM) as psrp,
            tc.tile_pool(name="cin", bufs=1, space="DRAM") as cin,
            tc.tile_pool(name="cout", bufs=1, space="DRAM") as cout,
        ):
            # ---------- resident tiles ----------
            vp = res.tile([128, NCH * CH], f16)          # tril diag packs, 2MB
            wa = res.tile([128, NWA * CH], f16)          # within+cross packs
            s0g = res.tile([128, KCH], bf16)
            s0f = res.tile([128, NCH], f16)
            ns0 = res.tile([128, NCH], f16)
            thn = res.tile([128, NCH], f32)              # -th
            outv = res.tile([128, NCH], f32)
            flags = res.tile([128, NCH], f32)
            e_t = res.tile([128, NCH], f16)              # e = s - s0 per chunk
            rowb = res.tile([1, NF], f32)                # row-layout margins
            ones = res.tile([1, 1], f32)
            trep = res.tile([128, 1], f32)

            nc.sync.dma_start(out=s0g[:, :], in_=s0g_t[:, :])

            # ---------- resident loads + thresholds (issued during the AG
            # window on the shard path; SP's ci-wait delays them past phase 1)
            def _late_loads():
                nc.sync.dma_start(out=vp[:, :], in_=vpack[:, :])
                nc.sync.dma_start(out=wa[:, :], in_=wapack[:, :])
                nc.sync.dma_start(out=s0f[:, :], in_=s0f_t[:, :])
                nc.sync.dma_start(out=ns0[:, :], in_=ns0f_t[:, :])
                nc.sync.dma_start(out=trep[:, :], in_=t_rep[:, :])
                nc.vector.memset(ones[:, :], 1.0)
                nc.vector.memset(outv[:, :], 0.0)
                nc.vector.memset(flags[:, :], 0.0)
                ut = res.tile([128, NCH], f32, name="ut")
                nc.sync.dma_start(out=ut[:, :], in_=u_t[:, :])
                lu = res.tile([128, NCH], f32, name="lu")
                om = res.tile([128, NCH], f32, name="om")
                nc.scalar.activation(lu[:, :], ut[:, :], AF.Ln)
                nc.vector.tensor_scalar(om[:, :], ut[:, :], -1.0, 1.0,
                                        AO.mult, AO.add)
                nc.scalar.activation(om[:, :], om[:, :], AF.Ln)
                nc.vector.tensor_tensor(out=om[:, :], in0=om[:, :],
                                        in1=lu[:, :], op=AO.subtract)
                nc.vector.tensor_scalar(thn[:, :], om[:, :], trep[:, 0:1],
                                        None, AO.mult)

            # ---------- phase 1: y0 rows ----------
            wtv = wt.ap().rearrange("(kc p) n -> kc p n", p=128)
            if shard:
                ps_row = rowp.tile([1, SUP], f32, tag="row")
                for k in range(KCH):
                    t = wtp.tile([128, 2 * SUP], bf16, name="t", tag="wt")
                    nc.sync.dma_start(out=t[:, :], in_=wtv[k, :, :])
                    for h in range(2):
                        nc.tensor.matmul(
                            ps_row[:, :], s0g[:, k:k + 1],
                            t[:, h * SUP:(h + 1) * SUP],
                            start=(k == 0 and h == 0),
                            stop=(k == KCH - 1 and h == 1))
                ib = smp.tile([1, SUP], f32, tag="ib")
                nc.scalar.activation(ib[:, :], ps_row[:, :], AF.Copy)
                ci = cin.tile([1, SUP], f32, tag="ci")
                co = cout.tile([CORES, SUP], f32, tag="co")
                nc.sync.dma_start(out=ci[:, :], in_=ib[:, :])
                nc.gpsimd.collective_compute(
                    "AllGather", AO.bypass,
                    replica_groups=[list(range(CORES))],
                    ins=[ci[:, :].opt()], outs=[co[:, :].opt()])
                _late_loads()
                nc.sync.dma_start(
                    out=rowb[:, :],
                    in_=co[:, :].rearrange("g n -> () (g n)"))
            else:
                for half in range(2):
                    ps_rows = []
                    for k in range(KCH):
                        t = wtp.tile([128, NF], bf16, name="t", tag="wt")
                        nc.sync.dma_start(
                            out=t[:, :],
                            in_=wtv[k, :, half * NF:(half + 1) * NF])
                        for gg in range(4):
                            if k == 0:
                                ps_rows.append(rowp.tile([1, SUP], f32,
                                                         name=f"psr{gg}",
                                                         tag="row"))
                            for h in range(2):
                                nc.tensor.matmul(
                                    ps_rows[gg][:, :], s0g[:, k:k + 1],
                                    t[:, (2 * gg + h) * SUP:(2 * gg + h + 1) * SUP],
                                    start=(k == 0 and h == 0),
                                    stop=(k == KCH - 1 and h == 1))
                    for gg in range(4):
                        g = half * 4 + gg
                        nc.scalar.activation(
                            rowb[0:1, g * SUP:(g + 1) * SUP],
                            ps_rows[gg][:, :], AF.Copy)

            if not shard:
                _late_loads()

            # ---------- main loop over supers ----------
            # Row-form phase-2 pieces handle only sources <= S-2 (emitted
            # interleaved into resolve gaps); the adjacent super's correction
            # is applied column-form inside each chunk's prep matmuls, so the
            # transposes for super S depend only on data ready one super early.
            def piece(src, g):
                at = atp.tile([128, CPS * SUP], f16, name="at", tag="at")
                for ksub in range(CPS):
                    r0 = src * SUP + ksub * CH
                    nc.sync.dma_start(
                        out=at[:, ksub * SUP:(ksub + 1) * SUP],
                        in_=atc[r0:r0 + CH, g * SUP:(g + 1) * SUP])
                tmp = rowp.tile([1, SUP], f32, name="tmp", tag="row")
                for ksub in range(CPS):
                    nc.tensor.matmul(
                        tmp[:, :],
                        e_t[:, src * CPS + ksub:src * CPS + ksub + 1],
                        at[:, ksub * SUP:(ksub + 1) * SUP],
                        start=(ksub == 0), stop=(ksub == CPS - 1))
                nc.vector.tensor_tensor(
                    out=rowb[0:1, g * SUP:(g + 1) * SUP],
                    in0=rowb[0:1, g * SUP:(g + 1) * SUP],
                    in1=tmp[:, :], op=AO.add)

            nsup_run = 0 if stop_after == "phase1" else NSUP
            pending = []          # deferred row-form pieces: (target g, emit)
            tb_tiles = {}

            def emit_tb(S):
                # transpose row base to columns; fold -th.  rowb[S] carries y0
                # plus all pieces from sources <= S-2; the S-1 part comes via
                # the wx column packs in the chunk preps.
                ps_t = pstp.tile([128, CPS], f32, tag="pt")
                for pch in range(CPS):
                    nc.tensor.matmul(
                        ps_t[:, pch:pch + 1],
                        rowb[0:1, S * SUP + pch * CH:S * SUP + (pch + 1) * CH],
                        ones[:, :], start=True, stop=True)
                tb = smp.tile([128, CPS], f32, name="tb", tag="tb")
                nc.vector.tensor_tensor(
                    out=tb[:, :], in0=ps_t[:, :],
                    in1=thn[:, S * CPS:(S + 1) * CPS], op=AO.add)
                tb_tiles[S] = tb

            def pop_piece():
                if pending:
                    pending.sort(key=lambda x: x[0])
                    pending.pop(0)[1]()

            for S in range(nsup_run):
                if S not in tb_tiles:
                    emit_tb(S)
                tb = tb_tiles[S]

                for pch in range(CPS):
                    c = S * CPS + pch
                    # prep: ps_w = Vc@(-s0_c) + prev-super cross packs + within
                    ps_w = pswp.tile([128, 1], f32, tag="pw")
                    last_src = (S >= 1 and pch == 0) or pch > 0
                    nc.tensor.matmul(ps_w[:, :], vp[:, c * CH:(c + 1) * CH],
                                     ns0[:, c:c + 1],
                                     start=True, stop=not last_src)
                    if S >= 1:
                        for srcp in range(CPS):
                            wi = _wx_index(S - 1, srcp, pch)
                            src = (S - 1) * CPS + srcp
                            nc.tensor.matmul(
                                ps_w[:, :], wa[:, wi * CH:(wi + 1) * CH],
                                e_t[:, src:src + 1], start=False,
                                stop=(srcp == CPS - 1 and pch == 0))
                    for srcp in range(pch):
                        wi = _wa_index(S, srcp, pch)
                        src = S * CPS + srcp
                        nc.tensor.matmul(ps_w[:, :],
                                         wa[:, wi * CH:(wi + 1) * CH],
                                         e_t[:, src:src + 1],
                                         start=False, stop=(srcp == pch - 1))
                    bias = smp.tile([128, 1], f32, tag="bias")
                    nc.vector.tensor_tensor(out=bias[:, :], in0=ps_w[:, :],
                                            in1=tb[:, pch:pch + 1], op=AO.add)
                    # rounds
                    cur = s0f[:, c:c + 1]
                    prev = None
                    for r in range(R):
                        ps_r = psrp.tile([128, 1], f32, name="ps_r", tag="pr")
                        nc.tensor.matmul(ps_r[:, :], vp[:, c * CH:(c + 1) * CH],
                                         cur, start=True, stop=True)
                        nxt = smp.tile([128, 1], f16, tag=f"s{r % 2}")
                        nc.scalar.activation(nxt[:, 0:1], ps_r[:, :], AF.Sign,
                                             bias=bias[:, 0:1])
                        prev = cur
                        cur = nxt[:, 0:1]
                    nc.vector.tensor_tensor(out=flags[:, c:c + 1], in0=cur,
                                            in1=prev, op=AO.subtract)
                    nc.vector.tensor_copy(outv[:, c:c + 1], cur)
                    nc.vector.tensor_tensor(out=e_t[:, c:c + 1], in0=cur,
                                            in1=s0f[:, c:c + 1],
                                            op=AO.subtract)
                    # fill PE gaps with the nearest-target deferred piece
                    pop_piece()
                    # hoist next super's transposes once its row base is final
                    # (all pieces targeting S+1 have been popped by chunk 1)
                    if pch == 1 and S + 1 < nsup_run and \
                            not any(g == S + 1 for g, _ in pending):
                        emit_tb(S + 1)
                # queue far-target row pieces of this super (sources S,
                # targets >= S+2) for emission inside the next super's gaps
                for g in range(S + 2, NSUP):
                    pending.append((g, lambda src=S, gg=g: piece(src, gg)))
            for _, fn_ in pending:
                fn_()

            nc.sync.dma_start(out=out_d[:, :], in_=outv[:, :])
            nc.sync.dma_start(out=flg_d[:, :], in_=flags[:, :])

    nc.compile()
    return nc


def _host_prep(w, initial_state, u, T, clamping_degree, perm, shard=SHARD):
    w = np.asarray(w, dtype=np.float32)
    s0 = np.asarray(initial_state, dtype=np.float32)
    u = np.asarray(u, dtype=np.float32)
    cd = np.asarray(clamping_degree)
    perm = np.asarray(perm).astype(np.int64)
    Tf = np.float32(T)

    fidx = np.where(cd[perm] == 0)[0]
    jf = perm[fidx]
    s0f = s0[jf]

    WT = np.ascontiguousarray(w[jf].T)       # [N, NF]  WT[k, n] = w[jf_n, k]
    ATc = np.ascontiguousarray(WT[jf, :].astype(np.float16))

    vpack = np.zeros((128, NCH * CH), dtype=np.float16)
    for c in range(NCH):
        blk = ATc[c * CH:(c + 1) * CH, c * CH:(c + 1) * CH]
        vpack[:, c * CH:(c + 1) * CH] = np.triu(blk, 1)

    wapack = np.zeros((128, NWA * CH), dtype=np.float16)
    for S in range(NSUP):
        for tgtp in range(1, CPS):
            for srcp in range(tgtp):
                wi = _wa_index(S, srcp, tgtp)
                src = S * CPS + srcp
                tgt = S * CPS + tgtp
                wapack[:, wi * CH:(wi + 1) * CH] = \
                    ATc[src * CH:(src + 1) * CH, tgt * CH:(tgt + 1) * CH]
    for Ssrc in range(NSUP - 1):
        for tgtp in range(CPS):
            for srcp in range(CPS):
                wi = _wx_index(Ssrc, srcp, tgtp)
                src = Ssrc * CPS + srcp
                tgt = (Ssrc + 1) * CPS + tgtp
                wapack[:, wi * CH:(wi + 1) * CH] = \
                    ATc[src * CH:(src + 1) * CH, tgt * CH:(tgt + 1) * CH]

    common = {
        "atc": ATc,
        "vpack": vpack,
        "wapack": wapack,
        "s0g_t": _tile_order(s0, KCH).astype(__import__("ml_dtypes").bfloat16),
        "s0f_t": _tile_order(s0f, NCH).astype(np.float16),
        "ns0f_t": _tile_order(-s0f, NCH).astype(np.float16),
        "u_t": _tile_order(u[fidx], NCH),
        "t_rep": np.full((128, 1), Tf, dtype=np.float32),
    }
    import ml_dtypes
    bf = ml_dtypes.bfloat16

    def hilo(block):
        # [N, C] fp32 -> [N, 2C] bf16 with per-SUP-column-group hi|lo halves
        C = block.shape[1]
        out = np.empty((block.shape[0], 2 * C), dtype=bf)
        hi = block.astype(bf)
        lo = (block - hi.astype(np.float32)).astype(bf)
        for g0 in range(0, C, SUP):
            out[:, 2 * g0:2 * g0 + SUP] = hi[:, g0:g0 + SUP]
            out[:, 2 * g0 + SUP:2 * g0 + 2 * SUP] = lo[:, g0:g0 + SUP]
        return np.ascontiguousarray(out)

    in_maps = []
    wt_full = None
    for r in range(CORES):
        m = dict(common)
        if shard:
            m["wt"] = hilo(WT[:, r * SUP:(r + 1) * SUP])
        else:
            if wt_full is None:
                wt_full = hilo(WT)
            m["wt"] = wt_full
        in_maps.append(m)
    return in_maps, {"jf": jf, "s0": s0}


_NC_CACHE = {}
LAST_RESULTS = None


def kernel(**inputs):
    global LAST_RESULTS
    from concourse.bass_utils import run_bass_kernel_spmd

    perm = np.asarray(inputs["perm"]).astype(np.int64)
    cd = np.asarray(inputs["clamping_degree"])
    is_perm = perm.shape == (N,) and (np.sort(perm) == np.arange(N)).all()
    if not is_perm or int((cd == 0).sum()) != NF:
        return _reference_fallback(**inputs)

    in_maps, meta = _host_prep(**inputs)
    trace = os.environ.get("KERNEL_TRACE", "0") == "1"

    for R in (R_ROUNDS, 10, 24):
        key = (R, SHARD)
        if key not in _NC_CACHE:
            _NC_CACHE[key] = _build_nc(R, SHARD)
        nc = _NC_CACHE[key]
        res = run_bass_kernel_spmd(nc, in_maps, core_ids=list(range(CORES)),
                                   trace=trace)
        LAST_RESULTS = res
        vals_t = res.results[0]["out_vals"]
        flags = res.results[0]["out_flags"]
        if float(np.abs(flags).sum()) == 0.0:
            break
    out = np.array(meta["s0"], dtype=np.float32, copy=True)
    out[meta["jf"]] = vals_t.T.reshape(-1)
    return out


def _reference_fallback(w, initial_state, u, T, clamping_degree, perm):
    """Generic exact numpy replay (only for unexpected input shapes)."""
    state = np.asarray(initial_state, dtype=np.float64).copy()
    w64 = np.asarray(w, dtype=np.float64)
    free = (np.asarray(clamping_degree) == 0)
    u64 = np.float64(np.asarray(u))
    th = float(T) * (np.log(u64) - np.log1p(-u64))
    for t in range(len(perm)):
        j = int(perm[t])
        if free[j]:
            s = w64[j] @ state
            state[j] = 1.0 if s >= th[t] else -1.0
    return state.astype(np.float32)


# revision 25
# speedup vs baseline: 1.3980x; 1.1598x over previous
"""Trainium2 Bass kernel for nn_BoltzmannMachine: one sequential Gibbs sweep
over N=8192 units (order `perm`), distributed over 8 NeuronCores.

Only the NF=4096 free units (clamping_degree==0) change; clamped units
contribute to every dot product only through the initial state.  Reduced
system (free steps i in perm order, unit j_i, threshold th_i = T*logit(u_i)):

    margin_i = w[j_i] . s0  -  th_i  +  sum_{l<i} A[i,l] * (s_l - s0_l)
    s_i = sign(margin_i),  A = w[jf][:, jf]

Device algorithm (per core, SPMD):
  phase 1   y0 rows via PE moving-operand matmuls (stationary = s0 chunk,
            moving = W^T tiles).  Row-sharded over the 8 cores (each core
            computes one 512-row super-block) + one AllGather.
  phase 2   cross-super corrections A[future, S] @ e_S (e = s - s0) as PE
            matmuls into PSUM rows, folded into the row base.
  resolve   per 128-step chunk: base = transpose(row base) - th + within-super
            corrections (PSUM col);  R fixed-point rounds
                s <- Sign( V_c @ s + bias )     (V_c = strict lower tri of A)
            one fp32 matmul + one ScalarE Sign per round.  Convergence flag
            (s_R != s_{R-1}) triggers a rerun with more rounds.

Host does data movement only on w (transpose/gather/re-layout); all O(N^2)
FLOPs and the sequential resolution run on device.
"""
import os
import numpy as np

N = 8192
NF = 4096
CH = 128
NCH = NF // CH          # 32 chunks
SUP = 512
NSUP = NF // SUP        # 8 super-blocks
CPS = SUP // CH         # 4 chunks per super
KCH = N // CH           # 64 global k-chunks
CORES = 8
R_ROUNDS = 5
SHARD = True            # phase-1 row-sharded + 1 AllGather


def _tile_order(vec, rt):
    """[128*rt] step-vector -> [128, rt] tile layout D[p, c] = vec[128*c + p]."""
    return np.ascontiguousarray(np.asarray(vec, np.float32).reshape(rt, 128).T)


NWA = 48 + 16 * (NSUP - 1)     # within-super + adjacent-cross packs


def _wa_index(S, srcp, tgtp):
    """Index of the within-super (src->tgt) pack, src<tgt."""
    tri = (tgtp * (tgtp - 1)) // 2 + srcp
    return S * 6 + tri


def _wx_index(Ssrc, srcp, tgtp):
    """Index of the cross pack (super Ssrc chunk srcp -> super Ssrc+1 chunk tgtp)."""
    return 48 + Ssrc * 16 + tgtp * CPS + srcp


def _build_nc(R=R_ROUNDS, shard=SHARD, stop_after=None):
    import concourse.bacc as bacc
    import concourse.bass as bass
    import concourse.mybir as mybir
    from concourse.tile import TileContext

    f32 = mybir.dt.float32
    f16 = mybir.dt.float16
    bf16 = mybir.dt.bfloat16
    AO = mybir.AluOpType
    AF = mybir.ActivationFunctionType

    nc = bacc.Bacc("TRN2", target_bir_lowering=False, debug=False,
                   num_devices=CORES)

    WTC = SUP if shard else NF
    wt = nc.declare_dram_parameter("wt", [N, 2 * WTC], bf16, isOutput=False)
    atc = nc.declare_dram_parameter("atc", [NF, NF], f16, isOutput=False)
    vpack = nc.declare_dram_parameter("vpack", [128, NCH * CH], f16,
                                      isOutput=False)
    wapack = nc.declare_dram_parameter("wapack", [128, NWA * CH], f16,
                                       isOutput=False)
    s0g_t = nc.declare_dram_parameter("s0g_t", [128, KCH], bf16, isOutput=False)
    s0f_t = nc.declare_dram_parameter("s0f_t", [128, NCH], f16, isOutput=False)
    ns0f_t = nc.declare_dram_parameter("ns0f_t", [128, NCH], f16, isOutput=False)
    u_t = nc.declare_dram_parameter("u_t", [128, NCH], f32, isOutput=False)
    t_rep = nc.declare_dram_parameter("t_rep", [128, 1], f32, isOutput=False)
    out_d = nc.declare_dram_parameter("out_vals", [128, NCH], f32, isOutput=True)
    flg_d = nc.declare_dram_parameter("out_flags", [128, NCH], f32,
                                      isOutput=True)

    with TileContext(nc) as tc:
        with (
            tc.tile_pool(name="res", bufs=1) as res,
            tc.tile_pool(name="wtp", bufs=6) as wtp,
            tc.tile_pool(name="atp", bufs=8) as atp,
            tc.tile_pool(name="sm", bufs=3) as smp,
            tc.tile_pool(name="rowp", bufs=(2 if shard else 4),
                         space=bass.MemorySpace.PSUM) as rowp,
            tc.tile_pool(name="pst", bufs=(2 if shard else 1),
                         space=bass.MemorySpace.PSUM) as pstp,
            tc.tile_pool(name="psw", bufs=(2 if shard else 1),
                         space=bass.MemorySpace.PSUM) as pswp,
            tc.tile_pool(name="psr", bufs=2, space=bass.MemorySpace.PSUM) as psrp,
            tc.tile_pool(name="cin", bufs=1, space="DRAM") as cin,
            tc.tile_pool(name="cout", bufs=1, space="DRAM") as cout,
        ):
            # ---------- resident tiles ----------
            vp = res.tile([128, NCH * CH], f16)          # tril diag packs, 2MB
            wa = res.tile([128, NWA * CH], f16)          # within+cross packs
            s0g = res.tile([128, KCH], bf16)
            s0f = res.tile([128, NCH], f16)
            ns0 = res.tile([128, NCH], f16)
            thn = res.tile([128, NCH], f32)              # -th
            outv = res.tile([128, NCH], f32)
            flags = res.tile([128, NCH], f32)
            e_t = res.tile([128, NCH], f16)              # e = s - s0 per chunk
            rowb = res.tile([1, NF], f32)                # row-layout margins
            ones = res.tile([1, 1], f32)
            trep = res.tile([128, 1], f32)

            nc.sync.dma_start(out=s0g[:, :], in_=s0g_t[:, :])

            # ---------- resident loads + thresholds (issued during the AG
            # window on the shard path; SP's ci-wait delays them past phase 1)
            def _late_loads():
                nc.sync.dma_start(out=vp[:, :], in_=vpack[:, :])
                nc.sync.dma_start(out=wa[:, :], in_=wapack[:, :])
                nc.sync.dma_start(out=s0f[:, :], in_=s0f_t[:, :])
                nc.sync.dma_start(out=ns0[:, :], in_=ns0f_t[:, :])
                nc.sync.dma_start(out=trep[:, :], in_=t_rep[:, :])
                nc.vector.memset(ones[:, :], 1.0)
                nc.vector.memset(outv[:, :], 0.0)
                nc.vector.memset(flags[:, :], 0.0)
                ut = res.tile([128, NCH], f32, name="ut")
                nc.sync.dma_start(out=ut[:, :], in_=u_t[:, :])
                lu = res.tile([128, NCH], f32, name="lu")
                om = res.tile([128, NCH], f32, name="om")
                nc.scalar.activation(lu[:, :], ut[:, :], AF.Ln)
                nc.vector.tensor_scalar(om[:, :], ut[:, :], -1.0, 1.0,
                                        AO.mult, AO.add)
                nc.scalar.activation(om[:, :], om[:, :], AF.Ln)
                nc.vector.tensor_tensor(out=om[:, :], in0=om[:, :],
                                        in1=lu[:, :], op=AO.subtract)
                nc.vector.tensor_scalar(thn[:, :], om[:, :], trep[:, 0:1],
                                        None, AO.mult)

            # ---------- phase 1: y0 rows ----------
            wtv = wt.ap().rearrange("(kc p) n -> kc p n", p=128)
            if shard:
                ps_row = rowp.tile([1, SUP], f32, tag="row")
                for k in range(KCH):
                    t = wtp.tile([128, 2 * SUP], bf16, name="t", tag="wt")
                    nc.sync.dma_start(out=t[:, :], in_=wtv[k, :, :])
                    for h in range(2):
                        nc.tensor.matmul(
                            ps_row[:, :], s0g[:, k:k + 1],
                            t[:, h * SUP:(h + 1) * SUP],
                            start=(k == 0 and h == 0),
                            stop=(k == KCH - 1 and h == 1))
                ib = smp.tile([1, SUP], f32, tag="ib")
                nc.scalar.activation(ib[:, :], ps_row[:, :], AF.Copy)
                ci = cin.tile([1, SUP], f32, tag="ci")
                co = cout.tile([CORES, SUP], f32, tag="co")
                nc.sync.dma_start(out=ci[:, :], in_=ib[:, :])
                nc.gpsimd.collective_compute(
                    "AllGather", AO.bypass,
                    replica_groups=[list(range(CORES))],
                    ins=[ci[:, :].opt()], outs=[co[:, :].opt()])
                _late_loads()
                nc.sync.dma_start(
                    out=rowb[:, :],
                    in_=co[:, :].rearrange("g n -> () (g n)"))
            else:
                for half in range(2):
                    ps_rows = []
                    for k in range(KCH):
                        t = wtp.tile([128, NF], bf16, name="t", tag="wt")
                        nc.sync.dma_start(
                            out=t[:, :],
                            in_=wtv[k, :, half * NF:(half + 1) * NF])
                        for gg in range(4):
                            if k == 0:
                                ps_rows.append(rowp.tile([1, SUP], f32,
                                                         name=f"psr{gg}",
                                                         tag="row"))
                            for h in range(2):
                                nc.tensor.matmul(
                                    ps_rows[gg][:, :], s0g[:, k:k + 1],
                                    t[:, (2 * gg + h) * SUP:(2 * gg + h + 1) * SUP],
                                    start=(k == 0 and h == 0),
                                    stop=(k == KCH - 1 and h == 1))
                    for gg in range(4):
                        g = half * 4 + gg
                        nc.scalar.activation(
                            rowb[0:1, g * SUP:(g + 1) * SUP],
                            ps_rows[gg][:, :], AF.Copy)

            if not shard:
                _late_loads()

            # ---------- main loop over supers ----------
            # Row-form phase-2 pieces handle only sources <= S-2 (emitted
            # interleaved into resolve gaps); the adjacent super's correction
            # is applied column-form inside each chunk's prep matmuls, so the
            # transposes for super S depend only on data ready one super early.
            def piece(src, g):
                at = atp.tile([128, CPS * SUP], f16, name="at", tag="at")
                for ksub in range(CPS):
                    r0 = src * SUP + ksub * CH
                    nc.sync.dma_start(
                        out=at[:, ksub * SUP:(ksub + 1) * SUP],
                        in_=atc[r0:r0 + CH, g * SUP:(g + 1) * SUP])
                tmp = rowp.tile([1, SUP], f32, name="tmp", tag="row")
                for ksub in range(CPS):
                    nc.tensor.matmul(
                        tmp[:, :],
                        e_t[:, src * CPS + ksub:src * CPS + ksub + 1],
                        at[:, ksub * SUP:(ksub + 1) * SUP],
                        start=(ksub == 0), stop=(ksub == CPS - 1))
                nc.vector.tensor_tensor(
                    out=rowb[0:1, g * SUP:(g + 1) * SUP],
                    in0=rowb[0:1, g * SUP:(g + 1) * SUP],
                    in1=tmp[:, :], op=AO.add)

            nsup_run = 0 if stop_after == "phase1" else NSUP
            pending = []          # deferred row-form pieces: (target g, emit)
            tb_tiles = {}

            def emit_tb(S):
                # transpose row base to columns; fold -th.  rowb[S] carries y0
                # plus all pieces from sources <= S-2; the S-1 part comes via
                # the wx column packs in the chunk preps.
                ps_t = pstp.tile([128, CPS], f32, tag="pt")
                for pch in range(CPS):
                    nc.tensor.matmul(
                        ps_t[:, pch:pch + 1],
                        rowb[0:1, S * SUP + pch * CH:S * SUP + (pch + 1) * CH],
                        ones[:, :], start=True, stop=True)
                tb = smp.tile([128, CPS], f32, name="tb", tag="tb")
                nc.vector.tensor_tensor(
                    out=tb[:, :], in0=ps_t[:, :],
                    in1=thn[:, S * CPS:(S + 1) * CPS], op=AO.add)
                tb_tiles[S] = tb

            def pop_piece():
                if pending:
                    pending.sort(key=lambda x: x[0])
                    pending.pop(0)[1]()

            for S in range(nsup_run):
                if S not in tb_tiles:
                    emit_tb(S)
                tb = tb_tiles[S]

                for pch in range(CPS):
                    c = S * CPS + pch
                    # prep: ps_w = Vc@(-s0_c) + prev-super cross packs + within
                    ps_w = pswp.tile([128, 1], f32, tag="pw")
                    last_src = (S >= 1 and pch == 0) or pch > 0
                    nc.tensor.matmul(ps_w[:, :], vp[:, c * CH:(c + 1) * CH],
                                     ns0[:, c:c + 1],
                                     start=True, stop=not last_src)
                    if S >= 1:
                        for srcp in range(CPS):
                            wi = _wx_index(S - 1, srcp, pch)
                            src = (S - 1) * CPS + srcp
                            nc.tensor.matmul(
                                ps_w[:, :], wa[:, wi * CH:(wi + 1) * CH],
                                e_t[:, src:src + 1], start=False,
                                stop=(srcp == CPS - 1 and pch == 0))
                    for srcp in range(pch):
                        wi = _wa_index(S, srcp, pch)
                        src = S * CPS + srcp
                        nc.tensor.matmul(ps_w[:, :],
                                         wa[:, wi * CH:(wi + 1) * CH],
                                         e_t[:, src:src + 1],
                                         start=False, stop=(srcp == pch - 1))
                    bias = smp.tile([128, 1], f32, tag="bias")
                    nc.vector.tensor_tensor(out=bias[:, :], in0=ps_w[:, :],
                                            in1=tb[:, pch:pch + 1], op=AO.add)
                    # rounds
                    cur = s0f[:, c:c + 1]
                    prev = None
                    for r in range(R):
                        ps_r = psrp.tile([128, 1], f32, name="ps_r", tag="pr")
                        nc.tensor.matmul(ps_r[:, :], vp[:, c * CH:(c + 1) * CH],
                                         cur, start=True, stop=True)
                        nxt = smp.tile([128, 1], f16, tag=f"s{r % 2}")
                        nc.scalar.activation(nxt[:, 0:1], ps_r[:, :], AF.Sign,
                                             bias=bias[:, 0:1])
                        prev = cur
                        cur = nxt[:, 0:1]
                    nc.vector.tensor_tensor(out=flags[:, c:c + 1], in0=cur,
                                            in1=prev, op=AO.subtract)
                    nc.vector.tensor_copy(outv[:, c:c + 1], cur)
                    nc.vector.tensor_tensor(out=e_t[:, c:c + 1], in0=cur,
                                            in1=s0f[:, c:c + 1],
                                            op=AO.subtract)
                    # fill PE gaps with the nearest-target deferred piece
                    pop_piece()
                    # hoist next super's transposes once its row base is final
                    # (all pieces targeting S+1 have been popped by chunk 1)
                    if pch == 1 and S + 1 < nsup_run and \
                            not any(g == S + 1 for g, _ in pending):
                        emit_tb(S + 1)
                # queue far-target row pieces of this super (sources S,
                # targets >= S+2) for emission inside the next super's gaps
                for g in range(S + 2, NSUP):
                    pending.append((g, lambda src=S, gg=g: piece(src, gg)))
            for _, fn_ in pending:
                fn_()

            nc.sync.dma_start(out=out_d[:, :], in_=outv[:, :])
            nc.sync.dma_start(out=flg_d[:, :], in_=flags[:, :])

    nc.compile()
    return nc


def _host_prep(w, initial_state, u, T, clamping_degree, perm, shard=SHARD):
    w = np.asarray(w, dtype=np.float32)
    s0 = np.asarray(initial_state, dtype=np.float32)
    u = np.asarray(u, dtype=np.float32)
    cd = np.asarray(clamping_degree)
    perm = np.asarray(perm).astype(np.int64)
    Tf = np.float32(T)

    fidx = np.where(cd[perm] == 0)[0]
    jf = perm[fidx]
    s0f = s0[jf]

    WT = np.ascontiguousarray(w[jf].T)       # [N, NF]  WT[k, n] = w[jf_n, k]
    ATc = np.ascontiguousarray(WT[jf, :].astype(np.float16))

    vpack = np.zeros((128, NCH * CH), dtype=np.float16)
    for c in range(NCH):
        blk = ATc[c * CH:(c + 1) * CH, c * CH:(c + 1) * CH]
        vpack[:, c * CH:(c + 1) * CH] = np.triu(blk, 1)

    wapack = np.zeros((128, NWA * CH), dtype=np.float16)
    for S in range(NSUP):
        for tgtp in range(1, CPS):
            for srcp in range(tgtp):
                wi = _wa_index(S, srcp, tgtp)
                src = S * CPS + srcp
                tgt = S * CPS + tgtp
                wapack[:, wi * CH:(wi + 1) * CH] = \
                    ATc[src * CH:(src + 1) * CH, tgt * CH:(tgt + 1) * CH]
    for Ssrc in range(NSUP - 1):
        for tgtp in range(CPS):
            for srcp in range(CPS):
                wi = _wx_index(Ssrc, srcp, tgtp)
                src = Ssrc * CPS + srcp
                tgt = (Ssrc + 1) * CPS + tgtp
                wapack[:, wi * CH:(wi + 1) * CH] = \
                    ATc[src * CH:(src + 1) * CH, tgt * CH:(tgt + 1) * CH]

    common = {
        "atc": ATc,
        "vpack": vpack,
        "wapack": wapack,
        "s0g_t": _tile_order(s0, KCH).astype(__import__("ml_dtypes").bfloat16),
        "s0f_t": _tile_order(s0f, NCH).astype(np.float16),
        "ns0f_t": _tile_order(-s0f, NCH).astype(np.float16),
        "u_t": _tile_order(u[fidx], NCH),
        "t_rep": np.full((128, 1), Tf, dtype=np.float32),
    }
    import ml_dtypes
    bf = ml_dtypes.bfloat16

    def hilo(block):
        # [N, C] fp32 -> [N, 2C] bf16 with per-SUP-column-group hi|lo halves
        C = block.shape[1]
        out = np.empty((block.shape[0], 2 * C), dtype=bf)
        hi = block.astype(bf)
        lo = (block - hi.astype(np.float32)).astype(bf)
        for g0 in range(0, C, SUP):
            out[:, 2 * g0:2 * g0 + SUP] = hi[:, g0:g0 + SUP]
            out[:, 2 * g0 + SUP:2 * g0 + 2 * SUP] = lo[:, g0:g0 + SUP]
        return np.ascontiguousarray(out)

    in_maps = []
    wt_full = None
    for r in range(CORES):
        m = dict(common)
        if shard:
            m["wt"] = hilo(WT[:, r * SUP:(r + 1) * SUP])
        else:
            if wt_full is None:
                wt_full = hilo(WT)
            m["wt"] = wt_full
        in_maps.append(m)
    return in_maps, {"jf": jf, "s0": s0}


_NC_CACHE = {}
LAST_RESULTS = None


def kernel(**inputs):
    global LAST_RESULTS
    from concourse.bass_utils import run_bass_kernel_spmd

    perm = np.asarray(inputs["perm"]).astype(np.int64)
    cd = np.asarray(inputs["clamping_degree"])
    is_perm = perm.shape == (N,) and (np.sort(perm) == np.arange(N)).all()
    if not is_perm or int((cd == 0).sum()) != NF:
        return _reference_fallback(**inputs)

    in_maps, meta = _host_prep(**inputs)
    trace = os.environ.get("KERNEL_TRACE", "0") == "1"

    converged = False
    for R in (R_ROUNDS, 10, 24):
        key = (R, SHARD)
        if key not in _NC_CACHE:
            _NC_CACHE[key] = _build_nc(R, SHARD)
        nc = _NC_CACHE[key]
        res = run_bass_kernel_spmd(nc, in_maps, core_ids=list(range(CORES)),
                                   trace=trace)
        LAST_RESULTS = res
        vals_t = res.results[0]["out_vals"]
        flags = res.results[0]["out_flags"]
        if float(np.abs(flags).sum()) == 0.0:
            converged = True
            break
    if not converged:
        return _reference_fallback(**inputs)
    out = np.array(meta["s0"], dtype=np.float32, copy=True)
    out[meta["jf"]] = vals_t.T.reshape(-1)
    return out


def _reference_fallback(w, initial_state, u, T, clamping_degree, perm):
    """Generic exact numpy replay (only for unexpected input shapes)."""
    state = np.asarray(initial_state, dtype=np.float64).copy()
    w64 = np.asarray(w, dtype=np.float64)
    free = (np.asarray(clamping_degree) == 0)
    u64 = np.float64(np.asarray(u))
    th = float(T) * (np.log(u64) - np.log1p(-u64))
    for t in range(len(perm)):
        j = int(perm[t])
        if free[j]:
            s = w64[j] @ state
            state[j] = 1.0 if s >= th[t] else -1.0
    return state.astype(np.float32)
